# revision 24
# baseline (speedup 1.0000x reference)
"""Trainium2 Bass kernel for a top-2 MoE layer (B=2, T=2048, D=1024, F=4096, E=8).

Strategy (expert-parallel, per sharding hint):
  Launch 1 (router, data-parallel over tokens): each of 8 cores computes
    logits = x_slice @ Wr in fp32 on the PE, then top-2 + renormalized
    softmax combine weights on-device (DVE/ACT).  Output: combine[4096, 8].
  Host dispatch (data movement only): tokens are gathered per expert
    (all-to-all performed by the host), padded to a static capacity.
  Launch 2 (expert FFN, expert-parallel): core e holds expert e's W1/W2
    (float32r for full-rate PE matmuls), computes y = c * (gelu(x@W1+b1)@W2
    + b2) for its gathered tokens.  F is processed in 4 quarter-passes so
    weights fit in SBUF; y accumulates in SBUF across passes.
  Launch 3 (combine): out[t] = yA[t] + yB[t] — the two selected experts'
    scaled outputs per token, added on-device, data-parallel over tokens.

All arithmetic is on-device; the host only reshapes/gathers/concats.
"""

import numpy as np

import concourse.bacc as bacc
import concourse.mybir as mybir
import concourse.tile as tile
from concourse import bass_utils

F32 = mybir.dt.float32
F32R = mybir.dt.float32r
AX = mybir.AxisListType
ALU = mybir.AluOpType
ACT_F = mybir.ActivationFunctionType

B, T, D, F, E = 2, 2048, 1024, 4096, 8
NTOK = B * T              # 4096
NCORES = 8
TOK_PER_CORE = NTOK // NCORES  # 512
FQ = F // 4               # F quarter = 1024
CHUNK = 512               # token chunk (moving-dim) for stage 1

_cache = {}


def _run(nc, in_maps, trace=False, **kw):
    return bass_utils.run_bass_kernel_spmd(
        nc, in_maps, core_ids=list(range(NCORES)), trace=trace, **kw
    )


# ----------------------------------------------------------------- router ---
def build_router():
    """Per core: xT_sl [D, 512] fp32, Wr [D, E] fp32 -> comb [512, E] fp32."""
    if "router" in _cache:
        return _cache["router"]
    nc = bacc.Bacc("TRN2", target_bir_lowering=False, debug=False)
    DO = D // 128  # 8 d-slices
    TT = TOK_PER_CORE // 128  # 4 token tiles
    # packed layouts: xT_sl[p, o*512+t] = x[tok0+t, o*128+p]; Wr[p, o*8+e]
    xT_d = nc.dram_tensor("xT_sl", [128, DO * TOK_PER_CORE], F32,
                          kind="ExternalInput").ap()
    wr_d = nc.dram_tensor("Wr", [128, DO * E], F32, kind="ExternalInput").ap()
    out_d = nc.dram_tensor("comb", [TOK_PER_CORE, E], F32, kind="ExternalOutput").ap()

    with tile.TileContext(nc) as tc:
        with (
            tc.tile_pool(name="pool", bufs=1) as pool,
            tc.tile_pool(name="work", bufs=2) as work,
            tc.tile_pool(name="psum", bufs=2, space="PSUM") as psum,
        ):
            xT_sb = pool.tile([128, DO, TOK_PER_CORE], F32)
            wr_sb = pool.tile([128, DO, E], F32)
            comb_sb = pool.tile([128, TT, E], F32)
            nc.gpsimd.dma_start(wr_sb[:], wr_d.rearrange("p (o e) -> p o e", o=DO))
            for dh in range(4):
                off = 2 * dh * TOK_PER_CORE
                nc.sync.dma_start(
                    xT_sb[:, 2 * dh:2 * dh + 2, :],
                    xT_d[:, off:off + 2 * TOK_PER_CORE].rearrange(
                        "p (o t) -> p o t", o=2),
                )

            for tt in range(TT):
                lp = psum.tile([128, E], F32)
                for do in range(DO):
                    nc.tensor.matmul(
                        lp[:],
                        xT_sb[:, do, tt * 128:(tt + 1) * 128],
                        wr_sb[:, do, :],
                        start=(do == 0),
                        stop=(do == DO - 1),
                    )
                l = work.tile([128, E], F32, tag="l")
                nc.vector.tensor_copy(l[:], lp[:])
                mx1 = work.tile([128, 1], F32, tag="mx1")
                nc.vector.reduce_max(mx1[:], l[:], axis=AX.X)
                nmx1 = work.tile([128, 1], F32, tag="nmx1")
                nc.vector.tensor_scalar_mul(nmx1[:], mx1[:], -1.0)
                eq = work.tile([128, E], F32, tag="eq")
                nc.vector.tensor_scalar(eq[:], l[:], mx1[:], None, op0=ALU.is_equal)
                lm = work.tile([128, E], F32, tag="lm")
                nc.vector.scalar_tensor_tensor(
                    lm[:], eq[:], -1e30, l[:], op0=ALU.mult, op1=ALU.add
                )
                mx2 = work.tile([128, 1], F32, tag="mx2")
                nc.vector.reduce_max(mx2[:], lm[:], axis=AX.X)
                p = work.tile([128, E], F32, tag="p")
                nc.scalar.activation(p[:], l[:], ACT_F.Exp, bias=nmx1[:])
                e2 = work.tile([128, 1], F32, tag="e2")
                nc.scalar.activation(e2[:], mx2[:], ACT_F.Exp, bias=nmx1[:])
                den = work.tile([128, 1], F32, tag="den")
                nc.vector.tensor_scalar_add(den[:], e2[:], 1.0)
                rec = work.tile([128, 1], F32, tag="rec")
                nc.vector.reciprocal(rec[:], den[:])
                ge = work.tile([128, E], F32, tag="ge")
                nc.vector.tensor_scalar(ge[:], l[:], mx2[:], None, op0=ALU.is_ge)
                w = work.tile([128, E], F32, tag="w")
                nc.vector.tensor_scalar_mul(w[:], p[:], rec[:])
                nc.vector.tensor_mul(comb_sb[:, tt, :], w[:], ge[:])

            nc.sync.dma_start(
                out_d.rearrange("(t p) e -> p t e", p=128), comb_sb[:]
            )
    nc.compile()
    _cache["router"] = nc
    return nc


# -------------------------------------------------------------------- ffn ---
def build_ffn(cap):
    """Per core (expert e): xTg [D, cap] f32r, W1 [D, F] f32r, b1 [F] f32,
    W2 [F, D] f32r, b2 [1, D] f32r, ones [1, 128] f32r, cvec [cap] f32
    -> y [cap, D] fp32 with y = cvec * (gelu(xg@W1 + b1) @ W2 + b2)."""
    key = ("ffn", cap)
    if key in _cache:
        return _cache[key]
    assert cap % 128 == 0
    DO = D // 128            # 8
    FT = FQ // 128           # 8 f-tiles per quarter pass
    TTILES = cap // 128
    # split cap into chunks of >=256 (fp32r full-rate) as evenly as possible
    k = -(-TTILES // 4)
    tile_counts = [TTILES // k + (1 if i < TTILES % k else 0) for i in range(k)]
    chunks = []
    c0 = 0
    for tc_ in tile_counts:
        chunks.append((c0, tc_ * 128))
        c0 += tc_ * 128
    CHUNKMAX = chunks[0][1]

    nc = bacc.Bacc("TRN2", target_bir_lowering=False, debug=False)
    # host pre-arranges weights/activations into SBUF layout so DMAs are
    # linear per partition:
    #   xTg[p, ci-flattened (o, t)]; W1h[p, q, o, f]; W2h[p, q, o, d]
    xT_d = nc.dram_tensor("xTg", [128, DO * cap], F32R, kind="ExternalInput").ap()
    w1_d = nc.dram_tensor("W1e", [128, 4, DO, FQ], F32R, kind="ExternalInput").ap()
    b1_d = nc.dram_tensor("b1e", [F], F32, kind="ExternalInput").ap()
    w2_d = nc.dram_tensor("W2e", [128, 4, FT, D], F32R, kind="ExternalInput").ap()
    b2_d = nc.dram_tensor("b2e", [1, D], F32R, kind="ExternalInput").ap()
    ones_d = nc.dram_tensor("ones", [1, 128], F32R, kind="ExternalInput").ap()
    cv_d = nc.dram_tensor("cvec", [cap], F32, kind="ExternalInput").ap()
    y_d = nc.dram_tensor("y", [cap, D], F32, kind="ExternalOutput").ap()

    with tile.TileContext(nc) as tc:
        with (
            tc.tile_pool(name="resident", bufs=1) as res,
            tc.tile_pool(name="w1p", bufs=1) as w1p,
            tc.tile_pool(name="w2p", bufs=2) as w2p,
            tc.tile_pool(name="xtp", bufs=2) as xtp,
            tc.tile_pool(name="htp", bufs=2) as htp,
            tc.tile_pool(name="ps1", bufs=3, space="PSUM") as ps1,
            tc.tile_pool(name="ps2", bufs=2, space="PSUM") as ps2,
            tc.tile_pool(name="psw", bufs=1, space="PSUM") as psw,
        ):
            y_acc = res.tile([128, TTILES, D], F32)
            b1_sb = res.tile([128, F // 128], F32)
            b2_sb = res.tile([1, D], F32R)
            ones_sb = res.tile([1, 128], F32R)
            cv_sb = res.tile([128, TTILES], F32)
            warm_sb = res.tile([128, 512], mybir.dt.bfloat16)
            nc.scalar.dma_start(b1_sb[:], b1_d.rearrange("(t p) -> p t", p=128))
            nc.scalar.dma_start(b2_sb[:], b2_d[:])
            nc.scalar.dma_start(ones_sb[:], ones_d[:])
            nc.scalar.dma_start(cv_sb[:], cv_d.rearrange("(t p) -> p t", p=128))

            # PE warmup while the first weight DMAs land (HAM ramp)
            nc.gpsimd.memset(warm_sb[:], 0.0)
            warm_ps = psw.tile([128, 512], F32)
            for _ in range(18):
                nc.tensor.matmul(warm_ps[:], warm_sb[:, :128], warm_sb[:],
                                 start=True, stop=True)

            def load_xt(ci, c0, cs):
                xT_sb = xtp.tile([128, DO, CHUNKMAX], F32R, tag="xt")
                off = DO * c0
                nc.sync.dma_start(
                    xT_sb[:, :, :cs],
                    xT_d[:, off:off + DO * cs].rearrange("p (o t) -> p o t", o=DO),
                )
                return xT_sb

            for q in range(4):
                w1_sb = w1p.tile([128, DO, FQ], F32R, tag="w1")
                w2_sb = w2p.tile([128, FT, D], F32R, tag="w2")

                def load_w1(dh):
                    nc.sync.dma_start(
                        w1_sb[:, 2 * dh:2 * dh + 2, :], w1_d[:, q, 2 * dh:2 * dh + 2, :]
                    )

                load_w1(0)
                xt0_sb = load_xt(0, *chunks[0]) if q == 0 else None
                for dh in range(1, 4):
                    load_w1(dh)
                for fh in range(2):
                    nc.gpsimd.dma_start(
                        w2_sb[:, 4 * fh:4 * fh + 4, :], w2_d[:, q, 4 * fh:4 * fh + 4, :]
                    )
                for ci, (c0, cs) in enumerate(chunks):
                    xT_sb = xt0_sb if (q == 0 and ci == 0) else load_xt(ci, c0, cs)
                    hT_sb = htp.tile([128, FT, CHUNKMAX], F32R, tag="ht")
                    # stage 1: hT[fq, tok] = gelu(W1q.T @ xT + b1)
                    for ft in range(FT):
                        hp = ps1.tile([128, CHUNKMAX], F32, tag="hp")
                        for do in range(DO):
                            nc.tensor.matmul(
                                hp[:, :cs],
                                w1_sb[:, do, ft * 128:(ft + 1) * 128],
                                xT_sb[:, do, :cs],
                                start=(do == 0),
                                stop=(do == DO - 1),
                            )
                        nc.scalar.activation(
                            hT_sb[:, ft, :cs], hp[:, :cs], ACT_F.Gelu,
                            bias=b1_sb[:, q * FT + ft:q * FT + ft + 1],
                        )
                    # stage 2: y[tok, d] (+)= hT.T @ W2q (+ b2 on last pass)
                    for tt in range(cs // 128):
                        gt = c0 // 128 + tt
                        yp = ps2.tile([128, D], F32, tag="yp")
                        for fo in range(FT):
                            for n in range(D // 512):
                                nc.tensor.matmul(
                                    yp[:, n * 512:(n + 1) * 512],
                                    hT_sb[:, fo, tt * 128:(tt + 1) * 128],
                                    w2_sb[:, fo, n * 512:(n + 1) * 512],
                                    start=(fo == 0),
                                    stop=(fo == FT - 1 and q != 3),
                                )
                        if q == 3:
                            for n in range(D // 512):
                                nc.tensor.matmul(
                                    yp[:, n * 512:(n + 1) * 512],
                                    ones_sb[:, :],
                                    b2_sb[:, n * 512:(n + 1) * 512],
                                    start=False,
                                    stop=True,
                                )
                        if q == 0:
                            nc.vector.tensor_scalar_mul(
                                y_acc[:, gt, :], yp[:], cv_sb[:, gt:gt + 1]
                            )
                        else:
                            nc.vector.scalar_tensor_tensor(
                                y_acc[:, gt, :], yp[:], cv_sb[:, gt:gt + 1],
                                y_acc[:, gt, :], op0=ALU.mult, op1=ALU.add,
                            )
                        if q == 3:
                            nc.sync.dma_start(
                                y_d.rearrange("(t p) d -> p t d", p=128)[:, gt, :],
                                y_acc[:, gt, :],
                            )
    nc.compile()
    _cache[key] = nc
    return nc


# ---------------------------------------------------------------- combine ---
def build_combine():
    """Per core: a, b [512, D] fp32 -> out [512, D] = a + b."""
    if "comb" in _cache:
        return _cache["comb"]
    nc = bacc.Bacc("TRN2", target_bir_lowering=False, debug=False)
    a_d = nc.dram_tensor("a", [TOK_PER_CORE, D], F32, kind="ExternalInput").ap()
    b_d = nc.dram_tensor("b", [TOK_PER_CORE, D], F32, kind="ExternalInput").ap()
    o_d = nc.dram_tensor("o", [TOK_PER_CORE, D], F32, kind="ExternalOutput").ap()
    with tile.TileContext(nc) as tc:
        with tc.tile_pool(name="pool", bufs=4) as pool:
            for tt in range(TOK_PER_CORE // 128):
                at = pool.tile([128, D], F32, tag="a")
                bt = pool.tile([128, D], F32, tag="b")
                for h in range(2):
                    hs = slice(h * 512, (h + 1) * 512)
                    nc.sync.dma_start(
                        at[:, hs], a_d.rearrange("(t p) d -> p t d", p=128)[:, tt, hs])
                    nc.gpsimd.dma_start(
                        bt[:, hs], b_d.rearrange("(t p) d -> p t d", p=128)[:, tt, hs])
                    nc.vector.tensor_add(at[:, hs], at[:, hs], bt[:, hs])
                    nc.sync.dma_start(
                        o_d.rearrange("(t p) d -> p t d", p=128)[:, tt, hs], at[:, hs])
    nc.compile()
    _cache["comb"] = nc
    return nc


# ----------------------------------------------------------------- driver ---
def _chunk_split(cap):
    ttiles = cap // 128
    k = -(-ttiles // 4)
    counts = [ttiles // k + (1 if i < ttiles % k else 0) for i in range(k)]
    chunks, c0 = [], 0
    for n in counts:
        chunks.append((c0, n * 128))
        c0 += n * 128
    return chunks


def _moe_forward(x2d, Wr, W1, b1, W2, b2, trace=False):
    """x2d: [NTOK, D] fp32. Returns (out [NTOK, D] fp32, exec_ns_total|None)."""
    DO = D // 128

    # --- launch 1: router ---
    rnc = build_router()
    wrh = np.ascontiguousarray(Wr.reshape(DO, 128, E).transpose(1, 0, 2).reshape(128, -1))
    in_maps = [
        {"xT_sl": np.ascontiguousarray(
            x2d[c * TOK_PER_CORE:(c + 1) * TOK_PER_CORE]
            .reshape(TOK_PER_CORE, DO, 128).transpose(2, 1, 0).reshape(128, -1)),
         "Wr": wrh}
        for c in range(NCORES)
    ]
    rres = _run(rnc, in_maps, trace=trace)
    comb = np.concatenate([rres.results[c]["comb"] for c in range(NCORES)], axis=0)
    exec_ns = rres.exec_time_ns or 0
    per_launch = [rres.exec_time_ns]

    # --- host dispatch (data movement only) ---
    top2 = np.argpartition(-comb, 1, axis=1)[:, :2]  # [NTOK, 2]
    sel_lists, cvals = [], []
    for e in range(E):
        sel = np.nonzero((top2 == e).any(axis=1))[0]
        sel_lists.append(sel)
        cvals.append(comb[sel, e])
    counts = np.array([len(s) for s in sel_lists])
    MAXCAP = 1664  # SBUF limit for y accumulator residency
    nbatch = max(1, -(-int(counts.max()) // MAXCAP))
    cap = int(max(128, -(-(-(-counts.max() // nbatch)) // 128) * 128))

    fnc = build_ffn(cap)
    chunks = _chunk_split(cap)
    ones_in = np.ones((1, 128), np.float32)
    w_packed = [
        {"W1e": np.ascontiguousarray(
            W1[e].reshape(DO, 128, 4, F // 4).transpose(1, 2, 0, 3)),
         "b1e": np.ascontiguousarray(b1[e]),
         "W2e": np.ascontiguousarray(
            W2[e].reshape(4, F // (4 * 128), 128, D).transpose(2, 0, 1, 3)),
         "b2e": np.ascontiguousarray(b2[e]).reshape(1, D)}
        for e in range(E)
    ]
    ys = [np.zeros((0, D), np.float32) for _ in range(E)]
    for bi in range(nbatch):
        in_maps = []
        for e in range(E):
            sel_b = sel_lists[e][bi * cap:(bi + 1) * cap]
            cv_b = cvals[e][bi * cap:(bi + 1) * cap]
            n_e = len(sel_b)
            xsel = np.zeros((cap, D), np.float32)
            xsel[:n_e] = x2d[sel_b]
            xg = np.concatenate(
                [xsel[c0:c0 + cs].reshape(cs, DO, 128).transpose(2, 1, 0)
                 .reshape(128, -1) for (c0, cs) in chunks], axis=1)
            cv = np.zeros(cap, np.float32)
            cv[:n_e] = cv_b
            in_maps.append({"xTg": np.ascontiguousarray(xg), "ones": ones_in,
                            "cvec": cv, **w_packed[e]})
        fres = _run(fnc, in_maps, trace=trace)
        ys = [np.concatenate([ys[e], fres.results[e]["y"]]) for e in range(E)]
        exec_ns += fres.exec_time_ns or 0
        per_launch.append(fres.exec_time_ns)

    # --- host: build per-token (A, B) contribution rows (gather only) ---
    slot = np.zeros((NTOK, E), np.int64)
    for e in range(E):
        slot[sel_lists[e], e] = np.arange(counts[e])
    e1, e2v = top2[:, 0], top2[:, 1]
    A = np.empty((NTOK, D), np.float32)
    Bm = np.empty((NTOK, D), np.float32)
    for e in range(E):
        m1 = e1 == e
        A[m1] = ys[e][slot[m1, e]]
        m2 = e2v == e
        Bm[m2] = ys[e][slot[m2, e]]

    # --- launch 3: combine ---
    cnc = build_combine()
    in_maps = [
        {"a": A[c * TOK_PER_CORE:(c + 1) * TOK_PER_CORE],
         "b": Bm[c * TOK_PER_CORE:(c + 1) * TOK_PER_CORE]}
        for c in range(NCORES)
    ]
    cres = _run(cnc, in_maps, trace=trace)
    out = np.concatenate([cres.results[c]["o"] for c in range(NCORES)], axis=0)
    exec_ns += cres.exec_time_ns or 0
    per_launch.append(cres.exec_time_ns)
    if trace:
        print(f"per-launch exec ns (router, ffn, combine): {per_launch}")
        _moe_forward.last = (rres, fres, cres)
    return out, (exec_ns if trace else None)


def kernel(x, Wr, W1, b1, W2, b2):
    x = np.asarray(x, np.float32)
    out, _ = _moe_forward(
        x.reshape(NTOK, D),
        np.asarray(Wr, np.float32),
        np.asarray(W1, np.float32),
        np.asarray(b1, np.float32),
        np.asarray(W2, np.float32),
        np.asarray(b2, np.float32),
        trace=False,
    )
    return out.reshape(B, T, D)


# revision 25
# speedup vs baseline: 1.0161x; 1.0161x over previous
"""Trainium2 Bass kernel for a top-2 MoE layer (B=2, T=2048, D=1024, F=4096, E=8).

Strategy (expert-parallel, per sharding hint):
  Launch 1 (router, data-parallel over tokens): each of 8 cores computes
    logits = x_slice @ Wr in fp32 on the PE, then top-2 + renormalized
    softmax combine weights on-device (DVE/ACT).  Output: combine[4096, 8].
  Host dispatch (data movement only): tokens are gathered per expert
    (all-to-all performed by the host), padded to a static capacity.
  Launch 2 (expert FFN, expert-parallel): core e holds expert e's W1/W2
    (float32r for full-rate PE matmuls), computes y = c * (gelu(x@W1+b1)@W2
    + b2) for its gathered tokens.  F is processed in 4 quarter-passes so
    weights fit in SBUF; y accumulates in SBUF across passes.
  Launch 3 (combine): out[t] = yA[t] + yB[t] — the two selected experts'
    scaled outputs per token, added on-device, data-parallel over tokens.

All arithmetic is on-device; the host only reshapes/gathers/concats.
"""

import numpy as np

import concourse.bacc as bacc
import concourse.mybir as mybir
import concourse.tile as tile
from concourse import bass_utils

F32 = mybir.dt.float32
F32R = mybir.dt.float32r
AX = mybir.AxisListType
ALU = mybir.AluOpType
ACT_F = mybir.ActivationFunctionType

B, T, D, F, E = 2, 2048, 1024, 4096, 8
NTOK = B * T              # 4096
NCORES = 8
TOK_PER_CORE = NTOK // NCORES  # 512
FQ = F // 4               # F quarter = 1024

_cache = {}


def _run(nc, in_maps, trace=False, **kw):
    return bass_utils.run_bass_kernel_spmd(
        nc, in_maps, core_ids=list(range(NCORES)), trace=trace, **kw
    )


# ----------------------------------------------------------------- router ---
def build_router():
    """Per core: xT_sl [D, 512] fp32, Wr [D, E] fp32 -> comb [512, E] fp32."""
    if "router" in _cache:
        return _cache["router"]
    nc = bacc.Bacc("TRN2", target_bir_lowering=False, debug=False)
    DO = D // 128  # 8 d-slices
    TT = TOK_PER_CORE // 128  # 4 token tiles
    # packed layouts: xT_sl[p, o*512+t] = x[tok0+t, o*128+p]; Wr[p, o*8+e]
    xT_d = nc.dram_tensor("xT_sl", [128, DO * TOK_PER_CORE], F32,
                          kind="ExternalInput").ap()
    wr_d = nc.dram_tensor("Wr", [128, DO * E], F32, kind="ExternalInput").ap()
    out_d = nc.dram_tensor("comb", [TOK_PER_CORE, E], F32, kind="ExternalOutput").ap()

    with tile.TileContext(nc) as tc:
        with (
            tc.tile_pool(name="pool", bufs=1) as pool,
            tc.tile_pool(name="work", bufs=2) as work,
            tc.tile_pool(name="psum", bufs=2, space="PSUM") as psum,
        ):
            xT_sb = pool.tile([128, DO, TOK_PER_CORE], F32)
            wr_sb = pool.tile([128, DO, E], F32)
            comb_sb = pool.tile([128, TT, E], F32)
            nc.gpsimd.dma_start(wr_sb[:], wr_d.rearrange("p (o e) -> p o e", o=DO))
            for dh in range(4):
                off = 2 * dh * TOK_PER_CORE
                nc.sync.dma_start(
                    xT_sb[:, 2 * dh:2 * dh + 2, :],
                    xT_d[:, off:off + 2 * TOK_PER_CORE].rearrange(
                        "p (o t) -> p o t", o=2),
                )

            for tt in range(TT):
                lp = psum.tile([128, E], F32)
                for do in range(DO):
                    nc.tensor.matmul(
                        lp[:],
                        xT_sb[:, do, tt * 128:(tt + 1) * 128],
                        wr_sb[:, do, :],
                        start=(do == 0),
                        stop=(do == DO - 1),
                    )
                l = work.tile([128, E], F32, tag="l")
                nc.vector.tensor_copy(l[:], lp[:])
                mx1 = work.tile([128, 1], F32, tag="mx1")
                nc.vector.reduce_max(mx1[:], l[:], axis=AX.X)
                nmx1 = work.tile([128, 1], F32, tag="nmx1")
                nc.vector.tensor_scalar_mul(nmx1[:], mx1[:], -1.0)
                eq = work.tile([128, E], F32, tag="eq")
                nc.vector.tensor_scalar(eq[:], l[:], mx1[:], None, op0=ALU.is_equal)
                lm = work.tile([128, E], F32, tag="lm")
                nc.vector.scalar_tensor_tensor(
                    lm[:], eq[:], -1e30, l[:], op0=ALU.mult, op1=ALU.add
                )
                mx2 = work.tile([128, 1], F32, tag="mx2")
                nc.vector.reduce_max(mx2[:], lm[:], axis=AX.X)
                p = work.tile([128, E], F32, tag="p")
                nc.scalar.activation(p[:], l[:], ACT_F.Exp, bias=nmx1[:])
                e2 = work.tile([128, 1], F32, tag="e2")
                nc.scalar.activation(e2[:], mx2[:], ACT_F.Exp, bias=nmx1[:])
                den = work.tile([128, 1], F32, tag="den")
                nc.vector.tensor_scalar_add(den[:], e2[:], 1.0)
                rec = work.tile([128, 1], F32, tag="rec")
                nc.vector.reciprocal(rec[:], den[:])
                ge = work.tile([128, E], F32, tag="ge")
                nc.vector.tensor_scalar(ge[:], l[:], mx2[:], None, op0=ALU.is_ge)
                w = work.tile([128, E], F32, tag="w")
                nc.vector.tensor_scalar_mul(w[:], p[:], rec[:])
                nc.vector.tensor_mul(comb_sb[:, tt, :], w[:], ge[:])

            nc.sync.dma_start(
                out_d.rearrange("(t p) e -> p t e", p=128), comb_sb[:]
            )
    nc.compile()
    _cache["router"] = nc
    return nc


# -------------------------------------------------------------------- ffn ---
def build_ffn(cap):
    """Per core (expert e): xTg [D, cap] f32r, W1 [D, F] f32r, b1 [F] f32,
    W2 [F, D] f32r, b2 [1, D] f32r, ones [1, 128] f32r, cvec [cap] f32
    -> y [cap, D] fp32 with y = cvec * (gelu(xg@W1 + b1) @ W2 + b2)."""
    key = ("ffn", cap)
    if key in _cache:
        return _cache[key]
    assert cap % 128 == 0
    DO = D // 128            # 8
    FT = FQ // 128           # 8 f-tiles per quarter pass
    TTILES = cap // 128
    # split cap into chunks of >=256 (fp32r full-rate) as evenly as possible
    k = -(-TTILES // 4)
    tile_counts = [TTILES // k + (1 if i < TTILES % k else 0) for i in range(k)]
    chunks = []
    c0 = 0
    for tc_ in tile_counts:
        chunks.append((c0, tc_ * 128))
        c0 += tc_ * 128
    CHUNKMAX = chunks[0][1]

    nc = bacc.Bacc("TRN2", target_bir_lowering=False, debug=False)
    # host pre-arranges weights/activations into SBUF layout so DMAs are
    # linear per partition:
    #   xTg[p, ci-flattened (o, t)]; W1h[p, q, o, f]; W2h[p, q, o, d]
    xT_d = nc.dram_tensor("xTg", [128, DO * cap], F32R, kind="ExternalInput").ap()
    w1_d = nc.dram_tensor("W1e", [128, 4, DO, FQ], F32R, kind="ExternalInput").ap()
    b1_d = nc.dram_tensor("b1e", [F], F32, kind="ExternalInput").ap()
    w2_d = nc.dram_tensor("W2e", [128, 4, FT, D], F32R, kind="ExternalInput").ap()
    b2_d = nc.dram_tensor("b2e", [1, D], F32R, kind="ExternalInput").ap()
    ones_d = nc.dram_tensor("ones", [1, 128], F32R, kind="ExternalInput").ap()
    cv_d = nc.dram_tensor("cvec", [cap], F32, kind="ExternalInput").ap()
    y_d = nc.dram_tensor("y", [cap, D], F32, kind="ExternalOutput").ap()

    with tile.TileContext(nc) as tc:
        with (
            tc.tile_pool(name="resident", bufs=1) as res,
            tc.tile_pool(name="w1p", bufs=1) as w1p,
            tc.tile_pool(name="w2p", bufs=2) as w2p,
            tc.tile_pool(name="xtp", bufs=2) as xtp,
            tc.tile_pool(name="htp", bufs=2) as htp,
            tc.tile_pool(name="ps1", bufs=3, space="PSUM") as ps1,
            tc.tile_pool(name="ps2", bufs=2, space="PSUM") as ps2,
            tc.tile_pool(name="psw", bufs=1, space="PSUM") as psw,
        ):
            y_acc = res.tile([128, TTILES, D], F32)
            b1_sb = res.tile([128, F // 128], F32)
            b2_sb = res.tile([1, D], F32R)
            ones_sb = res.tile([1, 128], F32R)
            cv_sb = res.tile([128, TTILES], F32)
            warm_sb = res.tile([128, 512], mybir.dt.bfloat16)
            nc.scalar.dma_start(b1_sb[:], b1_d.rearrange("(t p) -> p t", p=128))
            nc.scalar.dma_start(b2_sb[:], b2_d[:])
            nc.scalar.dma_start(ones_sb[:], ones_d[:])
            nc.scalar.dma_start(cv_sb[:], cv_d.rearrange("(t p) -> p t", p=128))

            # PE warmup while the first weight DMAs land (HAM ramp)
            nc.gpsimd.memset(warm_sb[:], 0.0)
            warm_ps = psw.tile([128, 512], F32)
            for _ in range(18):
                nc.tensor.matmul(warm_ps[:], warm_sb[:, :128], warm_sb[:],
                                 start=True, stop=True)

            def load_xt(ci, c0, cs):
                xT_sb = xtp.tile([128, DO, CHUNKMAX], F32R, tag="xt")
                off = DO * c0
                nc.sync.dma_start(
                    xT_sb[:, :, :cs],
                    xT_d[:, off:off + DO * cs].rearrange("p (o t) -> p o t", o=DO),
                )
                return xT_sb

            for q in range(4):
                w1_sb = w1p.tile([128, DO, FQ], F32R, tag="w1")
                w2_sb = w2p.tile([128, FT, D], F32R, tag="w2")

                def load_w1(dh):
                    nc.sync.dma_start(
                        w1_sb[:, 2 * dh:2 * dh + 2, :], w1_d[:, q, 2 * dh:2 * dh + 2, :]
                    )

                load_w1(0)
                xt0_sb = load_xt(0, *chunks[0]) if q == 0 else None
                for dh in range(1, 4):
                    load_w1(dh)
                for fh in range(2):
                    nc.gpsimd.dma_start(
                        w2_sb[:, 4 * fh:4 * fh + 4, :], w2_d[:, q, 4 * fh:4 * fh + 4, :]
                    )
                for ci, (c0, cs) in enumerate(chunks):
                    xT_sb = xt0_sb if (q == 0 and ci == 0) else load_xt(ci, c0, cs)
                    hT_sb = htp.tile([128, FT, CHUNKMAX], F32R, tag="ht")
                    # stage 1: hT[fq, tok] = gelu(W1q.T @ xT + b1)
                    for ft in range(FT):
                        hp = ps1.tile([128, CHUNKMAX], F32, tag="hp")
                        for do in range(DO):
                            nc.tensor.matmul(
                                hp[:, :cs],
                                w1_sb[:, do, ft * 128:(ft + 1) * 128],
                                xT_sb[:, do, :cs],
                                start=(do == 0),
                                stop=(do == DO - 1),
                            )
                        nc.scalar.activation(
                            hT_sb[:, ft, :cs], hp[:, :cs], ACT_F.Gelu,
                            bias=b1_sb[:, q * FT + ft:q * FT + ft + 1],
                        )
                    # stage 2: y[tok, d] (+)= hT.T @ W2q (+ b2 on last pass)
                    for tt in range(cs // 128):
                        gt = c0 // 128 + tt
                        yp = ps2.tile([128, D], F32, tag="yp")
                        for fo in range(FT):
                            for n in range(D // 512):
                                nc.tensor.matmul(
                                    yp[:, n * 512:(n + 1) * 512],
                                    hT_sb[:, fo, tt * 128:(tt + 1) * 128],
                                    w2_sb[:, fo, n * 512:(n + 1) * 512],
                                    start=(fo == 0),
                                    stop=(fo == FT - 1 and q != 3),
                                )
                        if q == 3:
                            for n in range(D // 512):
                                nc.tensor.matmul(
                                    yp[:, n * 512:(n + 1) * 512],
                                    ones_sb[:, :],
                                    b2_sb[:, n * 512:(n + 1) * 512],
                                    start=False,
                                    stop=True,
                                )
                        if q == 0:
                            nc.vector.tensor_scalar_mul(
                                y_acc[:, gt, :], yp[:], cv_sb[:, gt:gt + 1]
                            )
                        else:
                            nc.vector.scalar_tensor_tensor(
                                y_acc[:, gt, :], yp[:], cv_sb[:, gt:gt + 1],
                                y_acc[:, gt, :], op0=ALU.mult, op1=ALU.add,
                            )
                        if q == 3:
                            nc.sync.dma_start(
                                y_d.rearrange("(t p) d -> p t d", p=128)[:, gt, :],
                                y_acc[:, gt, :],
                            )
    nc.compile()
    _cache[key] = nc
    return nc


# ---------------------------------------------------------------- combine ---
def build_combine():
    """Per core: a, b [512, D] fp32 -> out [512, D] = a + b."""
    if "comb" in _cache:
        return _cache["comb"]
    nc = bacc.Bacc("TRN2", target_bir_lowering=False, debug=False)
    a_d = nc.dram_tensor("a", [TOK_PER_CORE, D], F32, kind="ExternalInput").ap()
    b_d = nc.dram_tensor("b", [TOK_PER_CORE, D], F32, kind="ExternalInput").ap()
    o_d = nc.dram_tensor("o", [TOK_PER_CORE, D], F32, kind="ExternalOutput").ap()
    with tile.TileContext(nc) as tc:
        with tc.tile_pool(name="pool", bufs=4) as pool:
            for tt in range(TOK_PER_CORE // 128):
                at = pool.tile([128, D], F32, tag="a")
                bt = pool.tile([128, D], F32, tag="b")
                for h in range(2):
                    hs = slice(h * 512, (h + 1) * 512)
                    nc.sync.dma_start(
                        at[:, hs], a_d.rearrange("(t p) d -> p t d", p=128)[:, tt, hs])
                    nc.gpsimd.dma_start(
                        bt[:, hs], b_d.rearrange("(t p) d -> p t d", p=128)[:, tt, hs])
                    nc.vector.tensor_add(at[:, hs], at[:, hs], bt[:, hs])
                    nc.sync.dma_start(
                        o_d.rearrange("(t p) d -> p t d", p=128)[:, tt, hs], at[:, hs])
    nc.compile()
    _cache["comb"] = nc
    return nc


# ----------------------------------------------------------------- driver ---
def _chunk_split(cap):
    ttiles = cap // 128
    k = -(-ttiles // 4)
    counts = [ttiles // k + (1 if i < ttiles % k else 0) for i in range(k)]
    chunks, c0 = [], 0
    for n in counts:
        chunks.append((c0, n * 128))
        c0 += n * 128
    return chunks


def _moe_forward(x2d, Wr, W1, b1, W2, b2, trace=False):
    """x2d: [NTOK, D] fp32. Returns (out [NTOK, D] fp32, exec_ns_total|None)."""
    DO = D // 128

    # --- launch 1: router ---
    rnc = build_router()
    wrh = np.ascontiguousarray(Wr.reshape(DO, 128, E).transpose(1, 0, 2).reshape(128, -1))
    in_maps = [
        {"xT_sl": np.ascontiguousarray(
            x2d[c * TOK_PER_CORE:(c + 1) * TOK_PER_CORE]
            .reshape(TOK_PER_CORE, DO, 128).transpose(2, 1, 0).reshape(128, -1)),
         "Wr": wrh}
        for c in range(NCORES)
    ]
    rres = _run(rnc, in_maps, trace=trace)
    comb = np.concatenate([rres.results[c]["comb"] for c in range(NCORES)], axis=0)
    exec_ns = rres.exec_time_ns or 0
    per_launch = [rres.exec_time_ns]

    # --- host dispatch (data movement only) ---
    top2 = np.argpartition(-comb, 1, axis=1)[:, :2]  # [NTOK, 2]
    sel_lists, cvals = [], []
    for e in range(E):
        sel = np.nonzero((top2 == e).any(axis=1))[0]
        sel_lists.append(sel)
        cvals.append(comb[sel, e])
    counts = np.array([len(s) for s in sel_lists])
    MAXCAP = 1664  # SBUF limit for y accumulator residency
    nbatch = max(1, -(-int(counts.max()) // MAXCAP))
    cap = int(max(128, -(-(-(-counts.max() // nbatch)) // 128) * 128))

    fnc = build_ffn(cap)
    chunks = _chunk_split(cap)
    ones_in = np.ones((1, 128), np.float32)
    w_packed = [
        {"W1e": np.ascontiguousarray(
            W1[e].reshape(DO, 128, 4, F // 4).transpose(1, 2, 0, 3)),
         "b1e": np.ascontiguousarray(b1[e]),
         "W2e": np.ascontiguousarray(
            W2[e].reshape(4, F // (4 * 128), 128, D).transpose(2, 0, 1, 3)),
         "b2e": np.ascontiguousarray(b2[e]).reshape(1, D)}
        for e in range(E)
    ]
    ys = [np.zeros((0, D), np.float32) for _ in range(E)]
    for bi in range(nbatch):
        in_maps = []
        for e in range(E):
            sel_b = sel_lists[e][bi * cap:(bi + 1) * cap]
            cv_b = cvals[e][bi * cap:(bi + 1) * cap]
            n_e = len(sel_b)
            xsel = np.zeros((cap, D), np.float32)
            xsel[:n_e] = x2d[sel_b]
            xg = np.concatenate(
                [xsel[c0:c0 + cs].reshape(cs, DO, 128).transpose(2, 1, 0)
                 .reshape(128, -1) for (c0, cs) in chunks], axis=1)
            cv = np.zeros(cap, np.float32)
            cv[:n_e] = cv_b
            in_maps.append({"xTg": np.ascontiguousarray(xg), "ones": ones_in,
                            "cvec": cv, **w_packed[e]})
        fres = _run(fnc, in_maps, trace=trace)
        ys = [np.concatenate([ys[e], fres.results[e]["y"]]) for e in range(E)]
        exec_ns += fres.exec_time_ns or 0
        per_launch.append(fres.exec_time_ns)

    # --- host: build per-token (A, B) contribution rows (gather only) ---
    slot = np.zeros((NTOK, E), np.int64)
    for e in range(E):
        slot[sel_lists[e], e] = np.arange(counts[e])
    e1, e2v = top2[:, 0], top2[:, 1]
    A = np.empty((NTOK, D), np.float32)
    Bm = np.empty((NTOK, D), np.float32)
    for e in range(E):
        m1 = e1 == e
        A[m1] = ys[e][slot[m1, e]]
        m2 = e2v == e
        Bm[m2] = ys[e][slot[m2, e]]

    # --- launch 3: combine ---
    cnc = build_combine()
    in_maps = [
        {"a": A[c * TOK_PER_CORE:(c + 1) * TOK_PER_CORE],
         "b": Bm[c * TOK_PER_CORE:(c + 1) * TOK_PER_CORE]}
        for c in range(NCORES)
    ]
    cres = _run(cnc, in_maps, trace=trace)
    out = np.concatenate([cres.results[c]["o"] for c in range(NCORES)], axis=0)
    exec_ns += cres.exec_time_ns or 0
    per_launch.append(cres.exec_time_ns)
    if trace:
        print(f"per-launch exec ns (router, ffn, combine): {per_launch}")
        _moe_forward.last = (rres, fres, cres)
    return out, (exec_ns if trace else None)


def kernel(x, Wr, W1, b1, W2, b2):
    x = np.asarray(x, np.float32)
    out, _ = _moe_forward(
        x.reshape(NTOK, D),
        np.asarray(Wr, np.float32),
        np.asarray(W1, np.float32),
        np.asarray(b1, np.float32),
        np.asarray(W2, np.float32),
        np.asarray(b2, np.float32),
        trace=False,
    )
    return out.reshape(B, T, D)
